# revision 18
# baseline (speedup 1.0000x reference)
"""Trainium2 Bass kernel for nn_AttentionModel (TSP-style pointer-attention decoder).

Sharding: pure data parallel, batch 128 -> 16 per core x 8 cores.

Per-core design (B=16, N=1000->1024 padded, D=128, H=8, 64 greedy decode steps):
  The three big per-step streams (compat, attention*V, logits) all contract
  against the embeddings. fp32 weight-loads on the PE are 4 cycles/column, so
  embeddings are stored as COMPENSATED bf16 SPLIT PAIRS (hi = bf16(x),
  lo = bf16(x - hi)) in both layouts, and every contraction runs as three
  bf16 matmuls accumulated in fp32 PSUM:
      x @ u  ~=  xh@uh + xh@ul + xl@uh      (error ~2^-24, preserves the
                                             exact greedy trajectory)
  Per-(b, chunk) structure (PE weight loads are per-b) with outputs in
  chunk-transposed layouts so softmax/argmax get all 128 partitions:
    compatT_c[n,(b,h)] = ehT/elT_chunk.T @ (uh|ul)      3 matmuls, FWL bf16
    row sums of exp via PE ones-matmuls (no partition reductions)
    E[(d),(b,h)]       = ehN/elN_chunk.T @ (xh|xl)      3 matmuls
    glimpseT = sum_h C_h.T @ E_h / se  (C = Wv_h @ W_out/sqrt(D), fp32)
    logitsT_c[n,b]     = ehT/elT_chunk.T @ (wh|wl)      3 matmuls
    tanh*10 and +/-inf visited mask in chunk-T layout, transpose back for
    per-b argmax (vector.max/max_index) and log-softmax.
  (b,h) columns indexed b*8+h.
"""

import numpy as np

import concourse.bass as bass
import concourse.mybir as mybir
from concourse.bacc import Bacc
from concourse.tile import TileContext

F32 = mybir.dt.float32
BF16 = mybir.dt.bfloat16
U32 = mybir.dt.uint32

BL = 16        # batch per core
N = 1000       # nodes
D = 128        # model dim
H = 8          # heads
DK = D // H    # 16
NCH = 8        # chunks of 128 per b
NPAD = 1024
NEG_BIG = -1.0e30
AluOp = mybir.AluOpType
ActFn = mybir.ActivationFunctionType


def build_program(S: int = 64) -> bass.Bass:
    nc = Bacc()

    emb = nc.dram_tensor("emb", [BL, NPAD, D], F32, kind="ExternalInput")
    WkT_d = nc.dram_tensor("WkT", [D, D], F32, kind="ExternalInput")
    WlT_d = nc.dram_tensor("WlT", [D, D], F32, kind="ExternalInput")
    Ws1_d = nc.dram_tensor("Ws1s", [D, D], F32, kind="ExternalInput")
    Ws2_d = nc.dram_tensor("Ws2s", [D, D], F32, kind="ExternalInput")
    Wfx_d = nc.dram_tensor("Wfxs", [D, D], F32, kind="ExternalInput")
    C_d = nc.dram_tensor("C_all", [D, H * D], F32, kind="ExternalInput")
    qp0_d = nc.dram_tensor("qp0s", [D, 1], F32, kind="ExternalInput")
    bmk_d = nc.dram_tensor("blockmask", [D, H], F32, kind="ExternalInput")
    iT2_d = nc.dram_tensor("iotaT2", [D, NPAD], F32, kind="ExternalInput")
    iTb_d = nc.dram_tensor("iotaTb", [D, D], F32, kind="ExternalInput")
    base_d = nc.dram_tensor("base_b", [BL, 1], U32, kind="ExternalInput")
    ones_d = nc.dram_tensor("ones", [D, 1], F32, kind="ExternalInput")
    onesb_d = nc.dram_tensor("ones_b16", [D, 1], BF16, kind="ExternalInput")
    onesr_d = nc.dram_tensor("ones_r", [1, D], F32, kind="ExternalInput")
    idn_d = nc.dram_tensor("ident", [D, D], F32, kind="ExternalInput")
    vneg0_d = nc.dram_tensor("visneg_init", [D, NPAD], F32, kind="ExternalInput")
    mout0_d = nc.dram_tensor("maskT_init", [D, D], F32, kind="ExternalInput")

    logp_o = nc.dram_tensor("logp_out", [BL, S, N], F32, kind="ExternalOutput")
    pi_o = nc.dram_tensor("pi_out", [BL, S], U32, kind="ExternalOutput")

    emb_flat = emb[:, :, :].rearrange("b n d -> (b n) d")

    with TileContext(nc) as tc:
        with (
            tc.tile_pool(name="res", bufs=1) as rp,
            tc.tile_pool(name="work", bufs=1) as wp,
            tc.tile_pool(name="ld", bufs=4) as lp,
            tc.tile_pool(name="psCT", bufs=2, space="PSUM") as pCT,
            tc.tile_pool(name="psE", bufs=1, space="PSUM") as pE,
            tc.tile_pool(name="psS", bufs=2, space="PSUM") as pS,
        ):
            # ---------------- residents: bf16 split pairs ----------------
            ehT = rp.tile([128, BL * NPAD], BF16)   # embT hi, col b*1024+n
            elT = rp.tile([128, BL * NPAD], BF16)   # embT lo
            ehN = rp.tile([128, BL * NCH * D], BF16)  # emb natural hi
            elN = rp.tile([128, BL * NCH * D], BF16)  # emb natural lo
            WkT = rp.tile_from(WkT_d[:, :])
            WlT = rp.tile_from(WlT_d[:, :])
            Ws1 = rp.tile_from(Ws1_d[:, :])
            Ws2 = rp.tile_from(Ws2_d[:, :])
            Wfx = rp.tile_from(Wfx_d[:, :])
            C_all = rp.tile_from(C_d[:, :])
            qp0 = rp.tile_from(qp0_d[:, :])
            bmk = rp.tile_from(bmk_d[:, :])
            iotaT2 = rp.tile_from(iT2_d[:, :])
            iotaTb = rp.tile_from(iTb_d[:, :])
            base_g = [rp.tile_from(base_d[0:8, :], name="base0"),
                      rp.tile_from(base_d[8:16, :], name="base1")]
            ones = rp.tile_from(ones_d[:, :])
            ones16 = rp.tile_from(onesb_d[:, :])
            ones_r = rp.tile_from(onesr_d[:, :])
            idn = rp.tile_from(idn_d[:, :])

            # ---------------- persistent state ----------------
            q_sb = wp.tile([128, BL], F32)
            fixed_sb = wp.tile([128, BL], F32)
            fixed2_sb = wp.tile([128, BL], F32)
            meanT = wp.tile([128, BL], F32)
            mean_parts = wp.tile([128, NCH * BL], F32)
            visnegT = wp.tile([128, NPAD], F32)
            mask_outT = wp.tile([128, D], F32)
            neginfT = wp.tile([128, D], F32)
            GL = 8  # batch items per pipeline half
            xf = wp.tile([128, NPAD], F32)          # exp fp32 (cols sliced per half)
            xh = wp.tile([128, NPAD], BF16)
            xl = wp.tile([128, NPAD], BF16)
            eqT = wp.tile([128, NPAD], F32)
            eqTb = wp.tile([128, D], mybir.dt.uint8)
            lt_sb = wp.tile([128, D], F32)
            Qbd = wp.tile([128, BL * H], F32)
            u_hl = wp.tile([128, BL * 2 * H], BF16)
            w_hl = wp.tile([128, BL * 2], BF16)
            E_sb = wp.tile([128, BL * H], F32)
            ise_bc = wp.tile([128, D], F32)
            expb = [wp.tile([128, GL * NCH], F32, name=f"expb{g}") for g in range(2)]
            masked_b = [wp.tile([GL, NPAD], F32, name=f"mb{g}") for g in range(2)]
            logp16 = [wp.tile([GL, NPAD], F32, name=f"lp{g}") for g in range(2)]
            gT_sb = [wp.tile([128, GL], F32, name=f"gT{g}") for g in range(2)]
            prev_nat = [wp.tile([GL, D], F32, name=f"pn{g}") for g in range(2)]
            prevT_sb = [wp.tile([128, GL], F32, name=f"pT{g}") for g in range(2)]
            max8 = [wp.tile([GL, 8], F32, name=f"m8{g}") for g in range(2)]
            sel8 = [wp.tile([GL, 8], U32, name=f"s8{g}") for g in range(2)]
            sel_f = [wp.tile([GL, 1], F32, name=f"sf{g}") for g in range(2)]
            sel_row = [wp.tile([1, GL], F32, name=f"sr{g}") for g in range(2)]
            ise_col = [wp.tile([64, 1], F32, name=f"ic{g}") for g in range(2)]
            ise_row = [wp.tile([1, 64], F32, name=f"ir{g}") for g in range(2)]
            se_b = [wp.tile([GL, 1], F32, name=f"seb{g}") for g in range(2)]
            lse = [wp.tile([GL, 1], F32, name=f"lse{g}") for g in range(2)]
            offs = [wp.tile([GL, 1], U32, name=f"of{g}") for g in range(2)]
            pi_sb = [wp.tile([GL, S], U32, name=f"pi{g}") for g in range(2)]

            # ---------------- init ----------------
            nc.sync.dma_start(out=visnegT[:, :], in_=vneg0_d[:, :])
            nc.sync.dma_start(out=mask_outT[:, :], in_=mout0_d[:, :])
            nc.vector.memset(neginfT[:, :], float("-inf"))

            # ---------------- precompute: load, split, transpose ----------
            for b in range(BL):
                for c in range(NCH):
                    col = (b * NCH + c) * D
                    tcol = b * NPAD + c * 128
                    tr = lp.tile([128, 128], F32, tag="tr")
                    nc.sync.dma_start(out=tr[:, :],
                                      in_=emb[b, c * 128:(c + 1) * 128, :])
                    # natural-layout splits
                    nc.scalar.copy(ehN[:, col:col + D], tr[:, :])
                    nc.vector.tensor_tensor(out=elN[:, col:col + D], in0=tr[:, :],
                                            in1=ehN[:, col:col + D],
                                            op=AluOp.subtract)
                    # transposed-layout splits (+ exact fp32 row sums for mean)
                    pt = pS.tile([128, 128], F32, tag="small")
                    nc.tensor.transpose(pt[:, :], tr[:, :], idn[:, :])
                    nc.scalar.activation(
                        out=ehT[:, tcol:tcol + 128], in_=pt[:, :], func=ActFn.Copy,
                        bias=0.0, scale=1.0,
                        accum_out=mean_parts[:, b * NCH + c:b * NCH + c + 1])
                    nc.vector.tensor_tensor(out=elT[:, tcol:tcol + 128],
                                            in0=pt[:, :],
                                            in1=ehT[:, tcol:tcol + 128],
                                            op=AluOp.subtract)
            for b in range(BL):
                nc.vector.tensor_reduce(
                    out=meanT[:, b:b + 1],
                    in_=mean_parts[:, b * NCH:(b + 1) * NCH],
                    axis=mybir.AxisListType.X, op=AluOp.add)

            fx_ps = pS.tile([128, 128], F32, tag="small")
            nc.tensor.matmul(fx_ps[0:D, 0:BL], lhsT=Wfx[:, :], rhs=meanT[:, :],
                             start=True, stop=True)
            nc.vector.tensor_copy(fixed_sb[:, :], fx_ps[0:D, 0:BL])
            nc.vector.tensor_tensor(
                out=q_sb[:, :], in0=fixed_sb[:, :],
                in1=qp0[:, 0:1].to_broadcast([128, BL]), op=AluOp.add)

            # ---------------- decode steps (two-half software pipeline) ----
            for s in range(S):
                for g in range(2):
                    bs, be = g * GL, (g + 1) * GL          # batch range
                    ph0, ph1 = g * 64, (g + 1) * 64        # (b,h) col range
                    # Qbd / U / U-splits for this half
                    nc.vector.tensor_tensor(
                        out=Qbd[:, ph0 * 1:ph1].rearrange("p (b j) -> p b j", j=H),
                        in0=q_sb[:, bs:be].unsqueeze(-1).to_broadcast([128, GL, H]),
                        in1=bmk[:, :].unsqueeze(1).to_broadcast([128, GL, H]),
                        op=AluOp.mult)
                    u_ps = pS.tile([128, 128], F32, tag="small")
                    nc.tensor.matmul(u_ps[:, 0:64], lhsT=WkT[:, :],
                                     rhs=Qbd[:, ph0:ph1], start=True, stop=True)
                    uv = u_hl[:, bs * 16:be * 16].rearrange(
                        "p (b t j) -> p b t j", t=2, j=H)
                    u_in = u_ps[:, 0:64].rearrange("p (b j) -> p b j", j=H)
                    nc.scalar.copy(uv[:, :, 0, :], u_in)
                    nc.vector.tensor_tensor(out=uv[:, :, 1, :], in0=u_in,
                                            in1=uv[:, :, 0, :], op=AluOp.subtract)

                    # compat chunks + exp splits + row sums
                    se_ps = pE.tile([64, 1], F32, tag="E")
                    for c in range(NCH):
                        ct = pCT.tile([128, 128], F32, tag="ct")
                        for bi in range(GL):
                            b = bs + bi
                            lhh = ehT[:, b * NPAD + c * 128: b * NPAD + (c + 1) * 128]
                            lhl = elT[:, b * NPAD + c * 128: b * NPAD + (c + 1) * 128]
                            nc.tensor.matmul(ct[:, bi * 16:(bi + 1) * 16], lhsT=lhh,
                                             rhs=u_hl[:, b * 16:(b + 1) * 16],
                                             start=True, stop=False)
                            nc.tensor.matmul(ct[:, bi * 16:bi * 16 + H], lhsT=lhl,
                                             rhs=u_hl[:, b * 16:b * 16 + H],
                                             start=False, stop=True)
                        cs = slice(c * 128 + ph0, c * 128 + ph1)
                        ct_r = ct[:, :].rearrange("p (b t j) -> p b t j", t=2, j=H)
                        xf_r = xf[:, cs].rearrange("p (b j) -> p b j", j=H)
                        nc.vector.tensor_tensor(
                            out=xf_r, in0=ct_r[:, :, 0, :],
                            in1=visnegT[:, cs].rearrange("p (b j) -> p b j", j=H),
                            op=AluOp.add)
                        nc.vector.tensor_tensor(out=xf_r, in0=ct_r[:, :, 1, :],
                                                in1=xf_r, op=AluOp.add)
                        nc.scalar.activation(out=xf[:, cs], in_=xf[:, cs],
                                             func=ActFn.Exp, bias=0.0, scale=1.0)
                        nc.scalar.copy(xh[:, cs], xf[:, cs])
                        nc.vector.tensor_tensor(out=xl[:, cs], in0=xf[:, cs],
                                                in1=xh[:, cs], op=AluOp.subtract)
                        nc.tensor.matmul(se_ps[0:64, 0:1], lhsT=xh[:, cs],
                                         rhs=ones16[:, 0:1],
                                         start=(c == 0), stop=False)
                        nc.tensor.matmul(se_ps[0:64, 0:1], lhsT=xl[:, cs],
                                         rhs=ones16[:, 0:1],
                                         start=False, stop=(c == NCH - 1))

                    nc.vector.reciprocal(ise_col[g][:, :], se_ps[0:64, :])
                    ir_ps = pS.tile([128, 128], F32, tag="small")
                    nc.tensor.transpose(ir_ps[0:1, 0:64], ise_col[g][:, 0:1],
                                        idn[0:64, 0:64])
                    nc.scalar.copy(ise_row[g][:, :], ir_ps[0:1, 0:64])
                    ib_ps = pS.tile([128, 128], F32, tag="small")
                    nc.tensor.matmul(ib_ps[:, 0:64], lhsT=ones_r[:, :],
                                     rhs=ise_row[g][0:1, :], start=True, stop=True)
                    nc.scalar.copy(ise_bc[:, ph0:ph1], ib_ps[:, 0:64])

                    # E = sum_n emb*attn (3-term), scaled by 1/se
                    E_ps = pE.tile([128, 64], F32, tag="E2")
                    for bi in range(GL):
                        b = bs + bi
                        for c in range(NCH):
                            lhh = ehN[:, (b * NCH + c) * D:(b * NCH + c + 1) * D]
                            lhl = elN[:, (b * NCH + c) * D:(b * NCH + c + 1) * D]
                            rh = xh[:, c * 128 + b * H: c * 128 + (b + 1) * H]
                            rl = xl[:, c * 128 + b * H: c * 128 + (b + 1) * H]
                            o = E_ps[:, bi * H:(bi + 1) * H]
                            nc.tensor.matmul(o, lhsT=lhh, rhs=rh,
                                             start=(c == 0), stop=False)
                            nc.tensor.matmul(o, lhsT=lhh, rhs=rl,
                                             start=False, stop=False)
                            nc.tensor.matmul(o, lhsT=lhl, rhs=rh,
                                             start=False, stop=(c == NCH - 1))
                    nc.vector.tensor_tensor(out=E_sb[:, ph0:ph1], in0=E_ps[:, 0:64],
                                            in1=ise_bc[:, ph0:ph1], op=AluOp.mult)

                    # glimpseT / w / w-splits
                    g_ps = pS.tile([128, 128], F32, tag="small")
                    E_r = E_sb[:, ph0:ph1].rearrange("p (b h) -> p h b", h=H)
                    for h in range(H):
                        nc.tensor.matmul(g_ps[0:D, 0:GL],
                                         lhsT=C_all[:, h * D:(h + 1) * D],
                                         rhs=E_r[:, h, :],
                                         start=(h == 0), stop=(h == H - 1))
                    nc.scalar.copy(gT_sb[g][:, :], g_ps[0:D, 0:GL])
                    w_ps = pS.tile([128, 128], F32, tag="small")
                    nc.tensor.matmul(w_ps[0:D, 0:GL], lhsT=WlT[:, :],
                                     rhs=gT_sb[g][:, :], start=True, stop=True)
                    wv = w_hl[:, bs * 2:be * 2].rearrange("p (b t) -> p b t", t=2)
                    nc.scalar.copy(wv[:, :, 0], w_ps[0:D, 0:GL])
                    nc.vector.tensor_tensor(out=wv[:, :, 1], in0=w_ps[0:D, 0:GL],
                                            in1=wv[:, :, 0], op=AluOp.subtract)

                    # logitsT chunks -> lt_sb cols c*16 + b
                    for c in range(NCH):
                        lt = pCT.tile([128, 128], F32, tag="ct")
                        for bi in range(GL):
                            b = bs + bi
                            lhh = ehT[:, b * NPAD + c * 128: b * NPAD + (c + 1) * 128]
                            lhl = elT[:, b * NPAD + c * 128: b * NPAD + (c + 1) * 128]
                            nc.tensor.matmul(lt[:, bi * 2:bi * 2 + 2], lhsT=lhh,
                                             rhs=w_hl[:, b * 2:b * 2 + 2],
                                             start=True, stop=False)
                            nc.tensor.matmul(lt[:, bi * 2:bi * 2 + 1], lhsT=lhl,
                                             rhs=w_hl[:, b * 2:b * 2 + 1],
                                             start=False, stop=True)
                        lts = lt_sb[:, c * BL + bs: c * BL + be]
                        lt_r = lt[:, 0:2 * GL].rearrange("p (b t) -> p b t", t=2)
                        nc.scalar.copy(lts, lt_r[:, :, 0])
                        nc.vector.tensor_tensor(out=lts, in0=lt_r[:, :, 1], in1=lts,
                                                op=AluOp.add)

                    # tanh*10 + visited mask in chunk-T layout (this half's cols)
                    ltv = lt_sb[:, :].rearrange("p (c b) -> p c b", b=BL)[:, :, bs:be]
                    mov = mask_outT[:, :].rearrange("p (c b) -> p c b", b=BL)[:, :, bs:be]
                    nc.scalar.activation(out=ltv, in_=ltv, func=ActFn.Tanh,
                                         bias=0.0, scale=1.0)
                    nc.vector.tensor_scalar(out=ltv, in0=ltv, scalar1=10.0,
                                            scalar2=None, op0=AluOp.mult)
                    nc.vector.tensor_tensor(out=ltv, in0=ltv, in1=mov, op=AluOp.min)

                    # log-sum-exp per b via PE column sums
                    nc.scalar.activation(
                        out=expb[g][:, :].rearrange("p (b c) -> p c b", c=NCH),
                        in_=ltv, func=ActFn.Exp, bias=0.0, scale=1.0)
                    sb_ps = pS.tile([128, 128], F32, tag="small")
                    for c in range(NCH):
                        nc.tensor.matmul(
                            sb_ps[0:GL, 0:1],
                            lhsT=expb[g][:, :].rearrange(
                                "p (b c) -> p c b", c=NCH)[:, c, :],
                            rhs=ones[:, 0:1],
                            start=(c == 0), stop=(c == NCH - 1))
                    nc.vector.tensor_copy(se_b[g][:, :], sb_ps[0:GL, 0:1])
                    nc.scalar.activation(out=lse[g][:, :], in_=se_b[g][:, :],
                                         func=ActFn.Ln, bias=0.0, scale=1.0)

                    # transpose masked logits back to [b, n] rows
                    for c in range(NCH):
                        mb_ps = pS.tile([128, 128], F32, tag="small")
                        nc.tensor.transpose(mb_ps[0:GL, 0:D],
                                            lt_sb[:, c * BL + bs: c * BL + be],
                                            idn[:, :])
                        nc.scalar.copy(masked_b[g][:, c * 128:(c + 1) * 128],
                                       mb_ps[0:GL, 0:D])

                    nc.vector.max(out=max8[g][:, :], in_=masked_b[g][:, :])
                    nc.vector.max_index(out=sel8[g][:, :], in_max=max8[g][:, :],
                                        in_values=masked_b[g][:, :])
                    nc.vector.tensor_scalar(out=logp16[g][:, :], in0=masked_b[g][:, :],
                                            scalar1=lse[g][:, 0:1], scalar2=None,
                                            op0=AluOp.subtract)
                    nc.sync.dma_start(out=logp_o[bs:be, s, :],
                                      in_=logp16[g][:, 0:N])
                    nc.vector.tensor_copy(pi_sb[g][:, s:s + 1], sel8[g][:, 0:1])

                    # ------- state updates for this half -------
                    nc.vector.tensor_copy(sel_f[g][:, :], sel8[g][:, 0:1])
                    sr_ps = pS.tile([128, 128], F32, tag="small")
                    nc.tensor.transpose(sr_ps[0:1, 0:GL], sel_f[g][0:GL, 0:1],
                                        idn[0:GL, 0:GL])
                    nc.scalar.copy(sel_row[g][:, :], sr_ps[0:1, 0:GL])
                    sbh_ps = pS.tile([128, 128], F32, tag="small")
                    nc.tensor.matmul(
                        sbh_ps[:, 0:64], lhsT=ones_r[:, :],
                        rhs=sel_row[g][0:1, :].unsqueeze(-1).to_broadcast([1, GL, H]),
                        start=True, stop=True)
                    eqv = eqT[:, :].rearrange("p (c x) -> p c x", x=128)[:, :, ph0:ph1]
                    nc.vector.tensor_tensor(
                        out=eqv,
                        in0=iotaT2[:, :].rearrange("p (c x) -> p c x", x=128)[:, :, ph0:ph1],
                        in1=sbh_ps[:, 0:64].unsqueeze(1).to_broadcast([128, NCH, 64]),
                        op=AluOp.is_equal)
                    sb2_ps = pS.tile([128, 128], F32, tag="small")
                    nc.tensor.matmul(sb2_ps[:, 0:GL], lhsT=ones_r[:, :],
                                     rhs=sel_row[g][0:1, :], start=True, stop=True)
                    eqbv = eqTb[:, :].rearrange("p (c x) -> p c x", x=BL)[:, :, bs:be]
                    nc.vector.tensor_tensor(
                        out=eqbv,
                        in0=iotaTb[:, :].rearrange("p (c x) -> p c x", x=BL)[:, :, bs:be],
                        in1=sb2_ps[:, 0:GL].unsqueeze(1).to_broadcast([128, NCH, GL]),
                        op=AluOp.is_equal)
                    nc.vector.copy_predicated(
                        out=mask_outT[:, :].rearrange("p (c b) -> p c b", b=BL)[:, :, bs:be],
                        mask=eqbv,
                        data=neginfT[:, :].rearrange("p (c b) -> p c b", b=BL)[:, :, bs:be])
                    nc.vector.tensor_scalar(out=eqv, in0=eqv, scalar1=NEG_BIG,
                                            scalar2=None, op0=AluOp.mult)
                    vnv = visnegT[:, :].rearrange("p (c x) -> p c x", x=128)[:, :, ph0:ph1]
                    nc.vector.tensor_tensor(out=vnv, in0=vnv, in1=eqv, op=AluOp.min)

                    # gather selected embeddings, next q (for this half)
                    nc.vector.tensor_tensor(out=offs[g][:, :], in0=sel8[g][:, 0:1],
                                            in1=base_g[g][:, :], op=AluOp.add)
                    nc.gpsimd.indirect_dma_start(
                        out=prev_nat[g][:, :], out_offset=None, in_=emb_flat,
                        in_offset=bass.IndirectOffsetOnAxis(ap=offs[g][:, 0:1], axis=0))
                    pv_ps = pS.tile([128, 128], F32, tag="small")
                    nc.tensor.transpose(pv_ps[0:D, 0:GL], prev_nat[g][:, :],
                                        idn[0:GL, 0:GL])
                    nc.scalar.copy(prevT_sb[g][:, :], pv_ps[0:D, 0:GL])
                    if s == 0:
                        f2_ps = pS.tile([128, 128], F32, tag="small")
                        nc.tensor.matmul(f2_ps[0:D, 0:GL], lhsT=Ws1[:, :],
                                         rhs=prevT_sb[g][:, :], start=True, stop=True)
                        nc.vector.tensor_tensor(out=fixed2_sb[:, bs:be],
                                                in0=fixed_sb[:, bs:be],
                                                in1=f2_ps[0:D, 0:GL], op=AluOp.add)
                    q_ps = pS.tile([128, 128], F32, tag="small")
                    nc.tensor.matmul(q_ps[0:D, 0:GL], lhsT=Ws2[:, :],
                                     rhs=prevT_sb[g][:, :], start=True, stop=True)
                    nc.vector.tensor_tensor(out=q_sb[:, bs:be], in0=fixed2_sb[:, bs:be],
                                            in1=q_ps[0:D, 0:GL], op=AluOp.add)

            nc.sync.dma_start(out=pi_o[0:8, :], in_=pi_sb[0][:, :])
            nc.sync.dma_start(out=pi_o[8:16, :], in_=pi_sb[1][:, :])

    nc.finalize()
    return nc


# ---------------------------------------------------------------------------
# Host-side input prep
# ---------------------------------------------------------------------------

def host_inputs(embeddings, W_node, W_fixed, W_step, W_out, W_placeholder,
                n_cores: int = 8):
    emb = np.ascontiguousarray(np.asarray(embeddings, np.float32))
    W_node = np.asarray(W_node, np.float32)
    W_fixed = np.asarray(W_fixed, np.float32)
    W_step = np.asarray(W_step, np.float32)
    W_out = np.asarray(W_out, np.float32)
    W_ph = np.asarray(W_placeholder, np.float32)
    B = emb.shape[0]
    inv_sdk = np.float32(1.0 / np.sqrt(DK))
    inv_sD = np.float32(1.0 / np.sqrt(D))

    Wk, Wv, Wl = W_node[:, :D], W_node[:, D:2 * D], W_node[:, 2 * D:]
    W_out_s = (W_out * inv_sD).astype(np.float32)
    C = np.concatenate(
        [(Wv[:, h * DK:(h + 1) * DK] @ W_out_s[h * DK:(h + 1) * DK, :]).astype(np.float32)
         for h in range(H)], axis=1)
    Ws1_s = np.ascontiguousarray((W_step[:D] * inv_sdk).astype(np.float32))
    Ws2_s = np.ascontiguousarray((W_step[D:] * inv_sdk).astype(np.float32))
    Wfx_s = (W_fixed * inv_sdk / np.float32(N)).astype(np.float32)
    qp0_s = ((W_ph @ W_step) * inv_sdk).astype(np.float32).reshape(D, 1)

    bmk = np.zeros((D, H), np.float32)
    for c in range(D):
        bmk[c, c // DK] = 1.0
    cc = (np.arange(NPAD) // 128).astype(np.float32)
    iotaT2 = (cc[None, :] * 128 + np.arange(D, dtype=np.float32)[:, None]).astype(np.float32)
    ccb = (np.arange(D) // BL).astype(np.float32)
    iotaTb = (ccb[None, :] * 128 + np.arange(D, dtype=np.float32)[:, None]).astype(np.float32)
    base = (np.arange(BL, dtype=np.uint32) * NPAD).reshape(BL, 1)
    vneg0 = np.zeros((D, NPAD), np.float32)
    vneg0[104:, (NCH - 1) * 128:] = NEG_BIG
    mout0 = np.full((D, D), np.inf, np.float32)
    mout0[104:, (NCH - 1) * BL:] = -np.inf

    import ml_dtypes
    consts = {
        "WkT": np.ascontiguousarray(Wk.T), "WlT": np.ascontiguousarray(Wl.T),
        "Ws1s": Ws1_s, "Ws2s": Ws2_s, "Wfxs": Wfx_s,
        "C_all": np.ascontiguousarray(C), "qp0s": qp0_s,
        "blockmask": bmk, "iotaT2": iotaT2, "iotaTb": iotaTb, "base_b": base,
        "ones": np.ones((D, 1), np.float32),
        "ones_b16": np.ones((D, 1), ml_dtypes.bfloat16),
        "ones_r": np.ones((1, D), np.float32),
        "ident": np.eye(D, dtype=np.float32),
        "visneg_init": vneg0, "maskT_init": mout0,
    }
    per_core = []
    bl = B // n_cores
    assert bl == BL
    for c in range(n_cores):
        m = dict(consts)
        ep = np.zeros((bl, NPAD, D), np.float32)
        ep[:, :N] = emb[c * bl:(c + 1) * bl]
        m["emb"] = ep
        per_core.append(m)
    return per_core


_prog_cache: dict = {}


def kernel(embeddings, W_node, W_fixed, W_step, W_out, W_placeholder, n_steps):
    from concourse import bass_utils

    S = int(n_steps)
    if S not in _prog_cache:
        _prog_cache[S] = build_program(S)
    nc = _prog_cache[S]

    in_maps = host_inputs(embeddings, W_node, W_fixed, W_step, W_out, W_placeholder)
    res = bass_utils.run_bass_kernel_spmd(nc, in_maps, core_ids=list(range(8)))
    logp = np.concatenate([r["logp_out"] for r in res.results], axis=0)
    pi = np.concatenate([r["pi_out"] for r in res.results], axis=0).astype(np.int32)
    return logp, pi


# revision 23
# speedup vs baseline: 1.5750x; 1.5750x over previous
"""Trainium2 Bass kernel for nn_AttentionModel (TSP-style pointer-attention decoder).

Sharding: pure data parallel, batch 128 -> 16 per core x 8 cores.

Per-core design (B=16, N=1000->1024 padded, D=128, H=8, 64 greedy decode steps):
  The three big per-step streams (compat, attention*V, logits) all contract
  against the embeddings. fp32 weight-loads on the PE are 4 cycles/column, so
  embeddings are stored as COMPENSATED bf16 SPLIT PAIRS (hi = bf16(x),
  lo = bf16(x - hi)) in both layouts, and every contraction runs as three
  bf16 matmuls accumulated in fp32 PSUM:
      x @ u  ~=  xh@uh + xh@ul + xl@uh      (error ~2^-24, preserves the
                                             exact greedy trajectory)
  Per-(b, chunk) structure (PE weight loads are per-b) with outputs in
  chunk-transposed layouts so softmax/argmax get all 128 partitions:
    compatT_c[n,(b,h)] = ehT/elT_chunk.T @ (uh|ul)      3 matmuls, FWL bf16
    row sums of exp via PE ones-matmuls (no partition reductions)
    E[(d),(b,h)]       = ehN/elN_chunk.T @ (xh|xl)      3 matmuls
    glimpseT = sum_h C_h.T @ E_h / se  (C = Wv_h @ W_out/sqrt(D), fp32)
    logitsT_c[n,b]     = ehT/elT_chunk.T @ (wh|wl)      3 matmuls
    tanh*10 and +/-inf visited mask in chunk-T layout, transpose back for
    per-b argmax (vector.max/max_index) and log-softmax.
  (b,h) columns indexed b*8+h.
"""

import numpy as np

import concourse.bass as bass
import concourse.mybir as mybir
from concourse.bacc import Bacc
from concourse.tile import TileContext

F32 = mybir.dt.float32
BF16 = mybir.dt.bfloat16
U32 = mybir.dt.uint32

BL = 16        # batch per core
N = 1000       # nodes
D = 128        # model dim
H = 8          # heads
DK = D // H    # 16
NCH = 8        # chunks of 128 per b
NPAD = 1024
NEG_BIG = -1.0e30
AluOp = mybir.AluOpType
ActFn = mybir.ActivationFunctionType


def build_program(S: int = 64) -> bass.Bass:
    nc = Bacc()

    emb = nc.dram_tensor("emb", [BL, NPAD, D], F32, kind="ExternalInput")
    WkT_d = nc.dram_tensor("WkT", [D, D], F32, kind="ExternalInput")
    WlT_d = nc.dram_tensor("WlT", [D, D], F32, kind="ExternalInput")
    Ws1_d = nc.dram_tensor("Ws1s", [D, D], F32, kind="ExternalInput")
    Ws2_d = nc.dram_tensor("Ws2s", [D, D], F32, kind="ExternalInput")
    Wfx_d = nc.dram_tensor("Wfxs", [D, D], F32, kind="ExternalInput")
    C_d = nc.dram_tensor("C_all", [D, H * D], F32, kind="ExternalInput")
    qp0_d = nc.dram_tensor("qp0s", [D, 1], F32, kind="ExternalInput")
    bmk_d = nc.dram_tensor("blockmask", [D, H], F32, kind="ExternalInput")
    iT2_d = nc.dram_tensor("iotaT2", [D, NPAD], F32, kind="ExternalInput")
    iTb_d = nc.dram_tensor("iotaTb", [D, D], F32, kind="ExternalInput")
    base_d = nc.dram_tensor("base_b", [BL, 1], U32, kind="ExternalInput")
    ones_d = nc.dram_tensor("ones", [D, 1], F32, kind="ExternalInput")
    onesb_d = nc.dram_tensor("ones_b16", [D, 1], BF16, kind="ExternalInput")
    onesr_d = nc.dram_tensor("ones_r", [1, D], F32, kind="ExternalInput")
    idn_d = nc.dram_tensor("ident", [D, D], F32, kind="ExternalInput")
    vneg0_d = nc.dram_tensor("visneg_init", [D, NPAD], F32, kind="ExternalInput")
    mout0_d = nc.dram_tensor("maskT_init", [D, D], F32, kind="ExternalInput")

    logp_o = nc.dram_tensor("logp_out", [BL, S, N], F32, kind="ExternalOutput")
    pi_o = nc.dram_tensor("pi_out", [BL, S], U32, kind="ExternalOutput")

    emb_flat = emb[:, :, :].rearrange("b n d -> (b n) d")

    with TileContext(nc) as tc:
        with (
            tc.tile_pool(name="res", bufs=1) as rp,
            tc.tile_pool(name="work", bufs=1) as wp,
            tc.tile_pool(name="ld", bufs=4) as lp,
            tc.tile_pool(name="psCT", bufs=3, space="PSUM") as pCT,
            tc.tile_pool(name="psE", bufs=1, space="PSUM") as pE,
            tc.tile_pool(name="psS", bufs=3, space="PSUM") as pS,
        ):
            # ---------------- residents: bf16 split pairs ----------------
            ehT = rp.tile([128, BL * NPAD], BF16)   # embT hi, col b*1024+n
            elT = rp.tile([128, BL * NPAD], BF16)   # embT lo
            ehN = rp.tile([128, BL * NCH * D], BF16)  # emb natural hi
            elN = rp.tile([128, BL * NCH * D], BF16)  # emb natural lo
            WkT = rp.tile_from(WkT_d[:, :])
            WlT = rp.tile_from(WlT_d[:, :])
            Ws1 = rp.tile_from(Ws1_d[:, :])
            Ws2 = rp.tile_from(Ws2_d[:, :])
            Wfx = rp.tile_from(Wfx_d[:, :])
            C_all = rp.tile_from(C_d[:, :])
            qp0 = rp.tile_from(qp0_d[:, :])
            bmk = rp.tile_from(bmk_d[:, :])
            iotaT2 = rp.tile_from(iT2_d[:, :])
            iotaTb = rp.tile_from(iTb_d[:, :])
            base = rp.tile_from(base_d[:, :])
            ones = rp.tile_from(ones_d[:, :])
            ones16 = rp.tile_from(onesb_d[:, :])
            ones_r = rp.tile_from(onesr_d[:, :])
            idn = rp.tile_from(idn_d[:, :])

            # ---------------- persistent state ----------------
            q_sb = wp.tile([128, BL], F32)
            fixed_sb = wp.tile([128, BL], F32)
            fixed2_sb = wp.tile([128, BL], F32)
            meanT = wp.tile([128, BL], F32)
            mean_parts = wp.tile([128, NCH * BL], F32)
            visnegT = wp.tile([128, NPAD], F32)
            mask_outT = wp.tile([128, D], F32)
            neginfT = wp.tile([128, D], F32)
            xf = wp.tile([128, NPAD], F32)          # exp fp32
            x_hl = wp.tile([128, 2 * NPAD], BF16)   # cols c*256+b*16+{0:8 hi,8:16 lo}
            eqT = wp.tile([128, NPAD], F32)
            eqTb = wp.tile([128, D], mybir.dt.uint8)
            lt_sb = wp.tile([128, D], F32)
            expb = wp.tile([128, D], F32)
            masked_b = wp.tile([BL, NPAD], F32)
            logp16 = wp.tile([BL, NPAD], F32)
            Qbd = wp.tile([128, BL * H], F32)
            u_hl = wp.tile([128, BL * 2 * H], BF16)   # cols b*16 + {0:8 hi, 8:16 lo}
            w_hl = wp.tile([128, BL * 2], BF16)       # cols b*2 + {0 hi, 1 lo}
            E_sb = wp.tile([128, BL * H], F32)
            E_f = wp.tile([128, BL * 2 * H], F32)
            gT_sb = wp.tile([128, BL], F32)
            w_sb = wp.tile([128, BL], F32)
            prev_nat = wp.tile([BL, D], F32)
            prevT_sb = wp.tile([128, BL], F32)
            max8 = wp.tile([BL, 8], F32)
            sel8 = wp.tile([BL, 8], U32)
            sel_f = wp.tile([BL, 1], F32)
            sel_row = wp.tile([1, BL], F32)
            ise_col = wp.tile([128, 1], F32)
            ise_row = wp.tile([1, D], F32)
            se_row = wp.tile([1, 2 * D], F32)
            ise_bc = wp.tile([128, D], F32)
            se_b = wp.tile([BL, 1], F32)
            lse = wp.tile([BL, 1], F32)
            offs = wp.tile([BL, 1], U32)
            pi_sb = wp.tile([BL, S], U32)

            # ---------------- init ----------------
            nc.sync.dma_start(out=visnegT[:, :], in_=vneg0_d[:, :])
            nc.sync.dma_start(out=mask_outT[:, :], in_=mout0_d[:, :])
            nc.vector.memset(neginfT[:, :], float("-inf"))

            # ---------------- precompute: load, split, transpose ----------
            for b in range(BL):
                for c in range(NCH):
                    col = (b * NCH + c) * D
                    tcol = b * NPAD + c * 128
                    tr = lp.tile([128, 128], F32, tag="tr")
                    nc.sync.dma_start(out=tr[:, :],
                                      in_=emb[b, c * 128:(c + 1) * 128, :])
                    # natural-layout splits
                    nc.scalar.copy(ehN[:, col:col + D], tr[:, :])
                    nc.vector.tensor_tensor(out=elN[:, col:col + D], in0=tr[:, :],
                                            in1=ehN[:, col:col + D],
                                            op=AluOp.subtract)
                    # transposed-layout splits (+ exact fp32 row sums for mean)
                    pt = pS.tile([128, 128], F32, tag="small")
                    nc.tensor.transpose(pt[:, :], tr[:, :], idn[:, :])
                    nc.scalar.activation(
                        out=ehT[:, tcol:tcol + 128], in_=pt[:, :], func=ActFn.Copy,
                        bias=0.0, scale=1.0,
                        accum_out=mean_parts[:, b * NCH + c:b * NCH + c + 1])
                    nc.vector.tensor_tensor(out=elT[:, tcol:tcol + 128],
                                            in0=pt[:, :],
                                            in1=ehT[:, tcol:tcol + 128],
                                            op=AluOp.subtract)
            for b in range(BL):
                nc.vector.tensor_reduce(
                    out=meanT[:, b:b + 1],
                    in_=mean_parts[:, b * NCH:(b + 1) * NCH],
                    axis=mybir.AxisListType.X, op=AluOp.add)

            fx_ps = pS.tile([128, 128], F32, tag="small")
            nc.tensor.matmul(fx_ps[0:D, 0:BL], lhsT=Wfx[:, :], rhs=meanT[:, :],
                             start=True, stop=True)
            nc.vector.tensor_copy(fixed_sb[:, :], fx_ps[0:D, 0:BL])
            nc.vector.tensor_tensor(
                out=q_sb[:, :], in0=fixed_sb[:, :],
                in1=qp0[:, 0:1].to_broadcast([128, BL]), op=AluOp.add)

            # ---------------- decode steps ----------------
            for s in range(S):
                # Qbd / U / U-splits
                nc.vector.tensor_tensor(
                    out=Qbd[:, :].rearrange("p (b j) -> p b j", j=H),
                    in0=q_sb[:, :].unsqueeze(-1).to_broadcast([128, BL, H]),
                    in1=bmk[:, :].unsqueeze(1).to_broadcast([128, BL, H]),
                    op=AluOp.mult)
                u_ps = pS.tile([128, 128], F32, tag="small")
                nc.tensor.matmul(u_ps[:, :], lhsT=WkT[:, :], rhs=Qbd[:, :],
                                 start=True, stop=True)
                u_hi = u_hl[:, :].rearrange("p (b t j) -> p b t j", t=2, j=H)[:, :, 0, :]
                u_lo = u_hl[:, :].rearrange("p (b t j) -> p b t j", t=2, j=H)[:, :, 1, :]
                u_in = u_ps[:, :].rearrange("p (b j) -> p b j", j=H)
                nc.scalar.copy(u_hi, u_in)
                nc.vector.tensor_tensor(out=u_lo, in0=u_in, in1=u_hi,
                                        op=AluOp.subtract)

                # compatT chunks + exp splits + row sums
                se_ps = pE.tile([1, 2 * 128], F32, tag="E")
                for c in range(NCH):
                    ct = pCT.tile([128, 2 * 128], F32, tag="ct")
                    for b in range(BL):
                        lhh = ehT[:, b * NPAD + c * 128: b * NPAD + (c + 1) * 128]
                        lhl = elT[:, b * NPAD + c * 128: b * NPAD + (c + 1) * 128]
                        nc.tensor.matmul(ct[:, b * 16:(b + 1) * 16], lhsT=lhh,
                                         rhs=u_hl[:, b * 16:(b + 1) * 16],
                                         start=True, stop=False)
                        nc.tensor.matmul(ct[:, b * 16:b * 16 + H], lhsT=lhl,
                                         rhs=u_hl[:, b * 16:b * 16 + H],
                                         start=False, stop=True)
                    cs = slice(c * 128, (c + 1) * 128)
                    ct_r = ct[:, :].rearrange("p (b t j) -> p b t j", t=2, j=H)
                    xf_r = xf[:, cs].rearrange("p (b j) -> p b j", j=H)
                    nc.vector.tensor_tensor(out=xf_r, in0=ct_r[:, :, 0, :],
                                            in1=visnegT[:, cs].rearrange(
                                                "p (b j) -> p b j", j=H),
                                            op=AluOp.add)
                    nc.vector.tensor_tensor(out=xf_r, in0=ct_r[:, :, 1, :],
                                            in1=xf_r, op=AluOp.add)
                    nc.scalar.activation(out=xf[:, cs], in_=xf[:, cs],
                                         func=ActFn.Exp, bias=0.0, scale=1.0)
                    xv = x_hl[:, c * 256:(c + 1) * 256].rearrange(
                        "p (b t j) -> p b t j", t=2, j=H)
                    xf_r = xf[:, cs].rearrange("p (b j) -> p b j", j=H)
                    nc.scalar.copy(xv[:, :, 0, :], xf_r)
                    nc.vector.tensor_tensor(out=xv[:, :, 1, :], in0=xf_r,
                                            in1=xv[:, :, 0, :], op=AluOp.subtract)
                    nc.tensor.matmul(se_ps[0:1, :], lhsT=ones16[:, 0:1],
                                     rhs=x_hl[:, c * 256:(c + 1) * 256],
                                     start=(c == 0), stop=(c == NCH - 1))

                nc.scalar.copy(se_row[0:1, :], se_ps[0:1, :])
                se_r = se_row[0:1, :].rearrange("p (b t j) -> p b t j", t=2, j=H)
                nc.vector.tensor_tensor(out=ise_row[0:1, :].rearrange(
                                            "p (b j) -> p b j", j=H),
                                        in0=se_r[:, :, 0, :], in1=se_r[:, :, 1, :],
                                        op=AluOp.add)
                nc.vector.reciprocal(ise_row[:, :], ise_row[:, :])
                ib_ps = pS.tile([128, 128], F32, tag="small")
                nc.tensor.matmul(ib_ps[:, 0:D], lhsT=ones_r[:, :],
                                 rhs=ise_row[0:1, :], start=True, stop=True)
                nc.scalar.copy(ise_bc[:, :], ib_ps[:, 0:D])

                # E = sum_n emb*attn (3-term), scaled by 1/se
                E_ps = pE.tile([128, BL * 2 * H], F32, tag="E")
                for b in range(BL):
                    for c in range(NCH):
                        lhh = ehN[:, (b * NCH + c) * D:(b * NCH + c + 1) * D]
                        lhl = elN[:, (b * NCH + c) * D:(b * NCH + c + 1) * D]
                        rhl = x_hl[:, c * 256 + b * 16: c * 256 + (b + 1) * 16]
                        nc.tensor.matmul(E_ps[:, b * 16:(b + 1) * 16], lhsT=lhh,
                                         rhs=rhl, start=(c == 0), stop=False)
                        nc.tensor.matmul(E_ps[:, b * 16:b * 16 + H], lhsT=lhl,
                                         rhs=rhl[:, 0:H],
                                         start=False, stop=(c == NCH - 1))
                nc.scalar.copy(E_f[:, :], E_ps[:, :])
                E_r2 = E_f[:, :].rearrange("p (b t j) -> p b t j", t=2, j=H)
                E_v = E_sb[:, :].rearrange("p (b j) -> p b j", j=H)
                nc.vector.tensor_tensor(out=E_v, in0=E_r2[:, :, 0, :],
                                        in1=E_r2[:, :, 1, :], op=AluOp.add)
                nc.vector.tensor_tensor(out=E_sb[:, :], in0=E_sb[:, :],
                                        in1=ise_bc[:, :], op=AluOp.mult)

                # glimpseT = sum_h C_h.T @ E[:, h::8]; w = Wl @ glimpseT; splits
                g_ps = pS.tile([128, 128], F32, tag="small")
                E_r = E_sb[:, :].rearrange("p (b h) -> p h b", h=H)
                for h in range(H):
                    nc.tensor.matmul(g_ps[0:D, 0:BL],
                                     lhsT=C_all[:, h * D:(h + 1) * D], rhs=E_r[:, h, :],
                                     start=(h == 0), stop=(h == H - 1))
                nc.scalar.copy(gT_sb[:, :], g_ps[0:D, 0:BL])
                w_ps = pS.tile([128, 128], F32, tag="small")
                nc.tensor.matmul(w_ps[0:D, 0:BL], lhsT=WlT[:, :], rhs=gT_sb[:, :],
                                 start=True, stop=True)
                w_hi = w_hl[:, :].rearrange("p (b t) -> p b t", t=2)[:, :, 0]
                w_lo = w_hl[:, :].rearrange("p (b t) -> p b t", t=2)[:, :, 1]
                nc.scalar.copy(w_hi, w_ps[0:D, 0:BL])
                nc.vector.tensor_tensor(out=w_lo, in0=w_ps[0:D, 0:BL], in1=w_hi,
                                        op=AluOp.subtract)

                # logitsT chunks (3-term) -> lt_sb[:, c*16+b]
                for c in range(NCH):
                    lt = pCT.tile([128, 2 * 128], F32, tag="ct")
                    for b in range(BL):
                        lhh = ehT[:, b * NPAD + c * 128: b * NPAD + (c + 1) * 128]
                        lhl = elT[:, b * NPAD + c * 128: b * NPAD + (c + 1) * 128]
                        nc.tensor.matmul(lt[:, b * 2:b * 2 + 2], lhsT=lhh,
                                         rhs=w_hl[:, b * 2:b * 2 + 2],
                                         start=True, stop=False)
                        nc.tensor.matmul(lt[:, b * 2:b * 2 + 1], lhsT=lhl,
                                         rhs=w_hl[:, b * 2:b * 2 + 1],
                                         start=False, stop=True)
                    lt_r = lt[:, 0:2 * BL].rearrange("p (b t) -> p b t", t=2)
                    nc.scalar.copy(lt_sb[:, c * BL:(c + 1) * BL], lt_r[:, :, 0])
                    nc.vector.tensor_tensor(out=lt_sb[:, c * BL:(c + 1) * BL],
                                            in0=lt_r[:, :, 1],
                                            in1=lt_sb[:, c * BL:(c + 1) * BL],
                                            op=AluOp.add)

                # tanh*10, visited mask (exact -inf) in chunk-T layout
                nc.scalar.activation(out=lt_sb[:, :], in_=lt_sb[:, :],
                                     func=ActFn.Tanh, bias=0.0, scale=1.0)
                nc.vector.tensor_scalar(out=lt_sb[:, :], in0=lt_sb[:, :],
                                        scalar1=10.0, scalar2=None, op0=AluOp.mult)
                nc.vector.tensor_tensor(out=lt_sb[:, :], in0=lt_sb[:, :],
                                        in1=mask_outT[:, :], op=AluOp.min)

                # log-sum-exp per b via PE column sums
                nc.scalar.activation(out=expb[:, :], in_=lt_sb[:, :],
                                     func=ActFn.Exp, bias=0.0, scale=1.0)
                sb_ps = pS.tile([128, 128], F32, tag="small")
                for c in range(NCH):
                    nc.tensor.matmul(sb_ps[0:BL, 0:1],
                                     lhsT=expb[:, c * BL:(c + 1) * BL],
                                     rhs=ones[:, 0:1],
                                     start=(c == 0), stop=(c == NCH - 1))
                nc.vector.tensor_copy(se_b[:, :], sb_ps[0:BL, 0:1])
                nc.scalar.activation(out=lse[:, :], in_=se_b[:, :], func=ActFn.Ln,
                                     bias=0.0, scale=1.0)

                # transpose masked logits back to [b, n] rows
                for c in range(NCH):
                    mb_ps = pS.tile([128, 128], F32, tag="small")
                    nc.tensor.transpose(mb_ps[0:BL, 0:D],
                                        lt_sb[:, c * BL:(c + 1) * BL], idn[:, :])
                    nc.scalar.copy(masked_b[:, c * 128:(c + 1) * 128],
                                   mb_ps[0:BL, 0:D])

                nc.vector.max(out=max8[:, :], in_=masked_b[:, :])
                nc.vector.max_index(out=sel8[:, :], in_max=max8[:, :],
                                    in_values=masked_b[:, :])
                nc.vector.tensor_scalar(out=logp16[:, :], in0=masked_b[:, :],
                                        scalar1=lse[:, 0:1], scalar2=None,
                                        op0=AluOp.subtract)
                nc.sync.dma_start(out=logp_o[:, s, :], in_=logp16[:, 0:N])
                nc.vector.tensor_copy(pi_sb[:, s:s + 1], sel8[:, 0:1])

                # ------- state updates -------
                nc.vector.tensor_copy(sel_f[:, :], sel8[:, 0:1])
                sr_ps = pS.tile([128, 128], F32, tag="small")
                nc.tensor.transpose(sr_ps[0:1, 0:BL], sel_f[0:BL, 0:1],
                                    idn[0:BL, 0:BL])
                nc.scalar.copy(sel_row[:, :], sr_ps[0:1, 0:BL])
                sbh_ps = pS.tile([128, 128], F32, tag="small")
                nc.tensor.matmul(
                    sbh_ps[:, 0:D], lhsT=ones_r[:, :],
                    rhs=sel_row[0:1, :].unsqueeze(-1).to_broadcast([1, BL, H]),
                    start=True, stop=True)
                nc.vector.tensor_tensor(
                    out=eqT[:, :].rearrange("p (c x) -> p c x", x=128),
                    in0=iotaT2[:, :].rearrange("p (c x) -> p c x", x=128),
                    in1=sbh_ps[:, 0:D].unsqueeze(1).to_broadcast([128, NCH, D]),
                    op=AluOp.is_equal)
                sb2_ps = pS.tile([128, 128], F32, tag="small")
                nc.tensor.matmul(sb2_ps[:, 0:BL], lhsT=ones_r[:, :],
                                 rhs=sel_row[0:1, :], start=True, stop=True)
                nc.vector.tensor_tensor(
                    out=eqTb[:, :].rearrange("p (c x) -> p c x", x=BL),
                    in0=iotaTb[:, :].rearrange("p (c x) -> p c x", x=BL),
                    in1=sb2_ps[:, 0:BL].unsqueeze(1).to_broadcast([128, NCH, BL]),
                    op=AluOp.is_equal)
                nc.vector.copy_predicated(out=mask_outT[:, :], mask=eqTb[:, :],
                                          data=neginfT[:, :])
                nc.vector.tensor_scalar(out=eqT[:, :], in0=eqT[:, :],
                                        scalar1=NEG_BIG, scalar2=None,
                                        op0=AluOp.mult)
                nc.vector.tensor_tensor(out=visnegT[:, :], in0=visnegT[:, :],
                                        in1=eqT[:, :], op=AluOp.min)

                # gather selected embeddings, next q
                nc.vector.tensor_tensor(out=offs[:, :], in0=sel8[:, 0:1],
                                        in1=base[:, :], op=AluOp.add)
                nc.gpsimd.indirect_dma_start(
                    out=prev_nat[:, :], out_offset=None, in_=emb_flat,
                    in_offset=bass.IndirectOffsetOnAxis(ap=offs[:, 0:1], axis=0))
                pv_ps = pS.tile([128, 128], F32, tag="small")
                nc.tensor.transpose(pv_ps[0:D, 0:BL], prev_nat[:, :],
                                    idn[0:BL, 0:BL])
                nc.scalar.copy(prevT_sb[:, :], pv_ps[0:D, 0:BL])
                if s == 0:
                    f2_ps = pS.tile([128, 128], F32, tag="small")
                    nc.tensor.matmul(f2_ps[0:D, 0:BL], lhsT=Ws1[:, :],
                                     rhs=prevT_sb[:, :], start=True, stop=True)
                    nc.vector.tensor_tensor(out=fixed2_sb[:, :], in0=fixed_sb[:, :],
                                            in1=f2_ps[0:D, 0:BL], op=AluOp.add)
                q_ps = pS.tile([128, 128], F32, tag="small")
                nc.tensor.matmul(q_ps[0:D, 0:BL], lhsT=Ws2[:, :], rhs=prevT_sb[:, :],
                                 start=True, stop=True)
                nc.vector.tensor_tensor(out=q_sb[:, :], in0=fixed2_sb[:, :],
                                        in1=q_ps[0:D, 0:BL], op=AluOp.add)

            nc.sync.dma_start(out=pi_o[:, :], in_=pi_sb[:, :])

    nc.finalize()
    return nc


# ---------------------------------------------------------------------------
# Host-side input prep
# ---------------------------------------------------------------------------

def host_inputs(embeddings, W_node, W_fixed, W_step, W_out, W_placeholder,
                n_cores: int = 8):
    emb = np.ascontiguousarray(np.asarray(embeddings, np.float32))
    W_node = np.asarray(W_node, np.float32)
    W_fixed = np.asarray(W_fixed, np.float32)
    W_step = np.asarray(W_step, np.float32)
    W_out = np.asarray(W_out, np.float32)
    W_ph = np.asarray(W_placeholder, np.float32)
    B = emb.shape[0]
    inv_sdk = np.float32(1.0 / np.sqrt(DK))
    inv_sD = np.float32(1.0 / np.sqrt(D))

    Wk, Wv, Wl = W_node[:, :D], W_node[:, D:2 * D], W_node[:, 2 * D:]
    W_out_s = (W_out * inv_sD).astype(np.float32)
    C = np.concatenate(
        [(Wv[:, h * DK:(h + 1) * DK] @ W_out_s[h * DK:(h + 1) * DK, :]).astype(np.float32)
         for h in range(H)], axis=1)
    Ws1_s = np.ascontiguousarray((W_step[:D] * inv_sdk).astype(np.float32))
    Ws2_s = np.ascontiguousarray((W_step[D:] * inv_sdk).astype(np.float32))
    Wfx_s = (W_fixed * inv_sdk / np.float32(N)).astype(np.float32)
    qp0_s = ((W_ph @ W_step) * inv_sdk).astype(np.float32).reshape(D, 1)

    bmk = np.zeros((D, H), np.float32)
    for c in range(D):
        bmk[c, c // DK] = 1.0
    cc = (np.arange(NPAD) // 128).astype(np.float32)
    iotaT2 = (cc[None, :] * 128 + np.arange(D, dtype=np.float32)[:, None]).astype(np.float32)
    ccb = (np.arange(D) // BL).astype(np.float32)
    iotaTb = (ccb[None, :] * 128 + np.arange(D, dtype=np.float32)[:, None]).astype(np.float32)
    base = (np.arange(BL, dtype=np.uint32) * NPAD).reshape(BL, 1)
    vneg0 = np.zeros((D, NPAD), np.float32)
    vneg0[104:, (NCH - 1) * 128:] = NEG_BIG
    mout0 = np.full((D, D), np.inf, np.float32)
    mout0[104:, (NCH - 1) * BL:] = -np.inf

    import ml_dtypes
    consts = {
        "WkT": np.ascontiguousarray(Wk.T), "WlT": np.ascontiguousarray(Wl.T),
        "Ws1s": Ws1_s, "Ws2s": Ws2_s, "Wfxs": Wfx_s,
        "C_all": np.ascontiguousarray(C), "qp0s": qp0_s,
        "blockmask": bmk, "iotaT2": iotaT2, "iotaTb": iotaTb, "base_b": base,
        "ones": np.ones((D, 1), np.float32),
        "ones_b16": np.ones((D, 1), ml_dtypes.bfloat16),
        "ones_r": np.ones((1, D), np.float32),
        "ident": np.eye(D, dtype=np.float32),
        "visneg_init": vneg0, "maskT_init": mout0,
    }
    per_core = []
    bl = B // n_cores
    assert bl == BL
    for c in range(n_cores):
        m = dict(consts)
        ep = np.zeros((bl, NPAD, D), np.float32)
        ep[:, :N] = emb[c * bl:(c + 1) * bl]
        m["emb"] = ep
        per_core.append(m)
    return per_core


_prog_cache: dict = {}


def kernel(embeddings, W_node, W_fixed, W_step, W_out, W_placeholder, n_steps):
    from concourse import bass_utils

    S = int(n_steps)
    if S not in _prog_cache:
        _prog_cache[S] = build_program(S)
    nc = _prog_cache[S]

    in_maps = host_inputs(embeddings, W_node, W_fixed, W_step, W_out, W_placeholder)
    res = bass_utils.run_bass_kernel_spmd(nc, in_maps, core_ids=list(range(8)))
    logp = np.concatenate([r["logp_out"] for r in res.results], axis=0)
    pi = np.concatenate([r["pi_out"] for r in res.results], axis=0).astype(np.int32)
    return logp, pi
